# revision 16
# baseline (speedup 1.0000x reference)
"""AssignAttention (topk_masking) Trainium2 kernel v2 — 8 NeuronCores.

Sharding: data-parallel over B (2 groups of 4 cores), tensor-parallel over
heads H (2 heads per core).

Math (per head): Y[n,s] = [n in top4 of column s of raw scores],
c_n = sum_s Y, cm_n = sum_s mask_s Y, e_n = exp(1/(c_n+1)), M = sum_s mask_s,
Z_n = M + (e_n-1) cm_n. Then
  out_head[n,:] = Vsum/Z_n + ((e_n-1)/Z_n) * (Y.mask @ v)[n,:]  (@ Wp rows)
which equals hard-topk + sum-normalize + masked softmax (see reference).

v2 structure vs baseline:
- Y' in {-1,+1} produced by Act-engine Sign((raw - midpoint(top4,top5))),
  with MAX8 reading raw straight from PSUM (no PSUM->SBUF evac of raw).
  Fixups: c = (c'+N)/2, cm = (cm'+M)/2, YmV = (Yv'+vs)/2 folded into the
  scalar math and the rank-one R term.
- counts matmuls run fp8 DoubleRow (lhsT padded to 128 wide).
- Final output: per-core n-slice via an 8-core AllToAll of the scaled
  T~ = (e-1)/(2Z) * Yv' matrices (fp8) + a tiny AllToAll of z/vsump rows;
  each core then computes out[nslice] = z2^T vsump + T~all^T Wp8 locally.
  No ReduceScatter chain.
"""
import sys, os

os.environ["JAX_ENABLE_COMPILATION_CACHE"] = "false"
sys.path.insert(0, "/opt/trn_rl_repo")
import numpy as np
import ml_dtypes

B, N, C, H, K = 2, 2048, 1024, 8, 4
HD = C // H
SCALE = HD ** -0.5
NCORES = 8
ST = 16            # 128-row s-tiles per head
KT = 12            # T~ fp8 prescale exponent
KP = 7             # Wp fp8 prescale exponent
KX = 6             # x fp8 prescale exponent (fp8 qk path)
KW = 12            # Wq/Wk fp8 prescale exponent
FP8_QK = False

f8t = ml_dtypes.float8_e4m3fn
_cache = {}


def _build():
    from concourse import bacc, tile, mybir

    f32, f16 = mybir.dt.float32, mybir.dt.float16
    f8 = mybir.dt.float8e4
    DR = mybir.MatmulPerfMode.DoubleRow
    AF = mybir.ActivationFunctionType
    OP = mybir.AluOpType
    GRP = [[0, 1, 2, 3, 4, 5, 6, 7]]

    nc = bacc.Bacc(None, target_bir_lowering=False)
    d_qt = nc.declare_dram_parameter("qt", [C, N], f16, isOutput=False)
    d_wq = nc.declare_dram_parameter("wq", [C, 2 * HD], f16, isOutput=False)
    d_wk = nc.declare_dram_parameter("wk", [C, 2 * HD], f16, isOutput=False)
    d_wv = nc.declare_dram_parameter("wv", [C, 2 * HD], f16, isOutput=False)
    d_wpo = nc.declare_dram_parameter("wpo", [128, 2 * C], f16, isOutput=False)
    d_wp8 = nc.declare_dram_parameter("wp8", [128, 16 * C], f8, isOutput=False)
    d_maskT = nc.declare_dram_parameter("maskT", [128, ST], f32, isOutput=False)
    d_om8 = nc.declare_dram_parameter("om8", [128, ST, 128], f8, isOutput=False)
    d_mb = nc.declare_dram_parameter("mbcol", [128, 1], f32, isOutput=False)
    d_ones = nc.declare_dram_parameter("onesc", [128, 16], f16, isOutput=False)
    d_ones8 = nc.declare_dram_parameter("onesc8", [128, 16], f8, isOutput=False)
    d_idt = nc.declare_dram_parameter("idt", [128, 128], f16, isOutput=False)
    d_idf = nc.declare_dram_parameter("idf", [128, 128], f32, isOutput=False)
    d_bqkv = nc.declare_dram_parameter("bqkv", [128, 6], f32, isOutput=False)
    d_gmask = nc.declare_dram_parameter("gmask", [16, 1], f32, isOutput=False)
    if FP8_QK:
        d_q8 = nc.declare_dram_parameter("q8", [C, N], f8, isOutput=False)
        d_wq8 = nc.declare_dram_parameter("wq8", [128, 8, 2 * HD], f8,
                                          isOutput=False)
        d_wk8 = nc.declare_dram_parameter("wk8", [128, 8, 2 * HD], f8,
                                          isOutput=False)
    d_out = nc.declare_dram_parameter("out", [N // 4, C], f16, isOutput=True)

    with tile.TileContext(nc) as tc:
        with (
            tc.tile_pool(name="cst", bufs=1) as cst,
            tc.tile_pool(name="ld", bufs=1) as ld,
            tc.tile_pool(name="proj", bufs=1) as proj,
            tc.tile_pool(name="work", bufs=2) as work,
            tc.tile_pool(name="outp", bufs=2) as outp,
            tc.tile_pool(name="psr", bufs=3, space="PSUM") as psr,
            tc.tile_pool(name="psc", bufs=1, space="PSUM") as psc,
            tc.tile_pool(name="psy", bufs=1, space="PSUM") as psy,
            tc.tile_pool(name="dram", bufs=1, space="DRAM") as dram,
        ):
            QS = [nc.sync, nc.scalar, nc.gpsimd]

            # ---------------- input DMAs ----------------
            # consts first (tiny), then qt chunks + weights, tail weights last
            if FP8_QK:
                q8_sb = ld.tile([128, 8, N], f8)
                for ci in range(8):
                    QS[ci % 3].dma_start(q8_sb[:, ci, :],
                                         d_q8[ci * 128:(ci + 1) * 128, :])
                wq8_sb = ld.tile([128, 8, 256], f8)
                wk8_sb = ld.tile([128, 8, 256], f8)
                nc.sync.dma_start(wq8_sb[:], d_wq8[:])
                nc.scalar.dma_start(wk8_sb[:], d_wk8[:])
            maskT = cst.tile([128, ST], f32)
            om8 = cst.tile([128, ST, 128], f8)
            mbcol = cst.tile([128, 1], f32)
            onesc = cst.tile([128, 16], f16)
            onesc8 = cst.tile([128, 16], f8)
            idt = cst.tile([128, 128], f16)
            idf = cst.tile([128, 128], f32)
            bqkv = cst.tile([128, 6], f32)
            gmask = cst.tile([16, 1], f32)
            for i, (t, d) in enumerate([(maskT, d_maskT), (mbcol, d_mb),
                                        (onesc, d_ones), (onesc8, d_ones8),
                                        (idt, d_idt),
                                        (idf, d_idf), (bqkv, d_bqkv),
                                        (gmask, d_gmask)]):
                QS[i % 3].dma_start(t[:], d[:])
            qt_sb = ld.tile([128, 8 * N], f16)
            for ci in range(8):
                for j in range(4):
                    QS[(4 * ci + j) % 3].dma_start(
                        qt_sb[:, ci * N + j * 512:ci * N + (j + 1) * 512],
                        d_qt[ci * 128:(ci + 1) * 128,
                             j * 512:(j + 1) * 512])
            w_sb = {}
            for nm, dd in (("q", d_wq), ("k", d_wk), ("v", d_wv)):
                w_sb[nm] = ld.tile([128, 8 * 2 * HD], f16, tag=f"w{nm}",
                                   name=f"wsb{nm}")
                for ci in range(8):
                    QS[(ci + 1) % 3].dma_start(
                        w_sb[nm][:, ci * 256:(ci + 1) * 256],
                        dd[ci * 128:(ci + 1) * 128, :])
            nc.sync.dma_start(om8[:], d_om8[:])
            wpo_sb = cst.tile([128, 2 * C], f16)
            nc.gpsimd.dma_start(wpo_sb[:], d_wpo[:])
            wp8_sb = cst.tile([128, 16, C], f8)
            for i in range(4):
                QS[i % 3].dma_start(wp8_sb[:, 4 * i:4 * i + 4, :],
                                    d_wp8[:, 4 * i * C:(4 * i + 4) * C])

            # ---------------- collective warmup ----------------
            dumA = dram.tile([8, 512], f16)
            dumO = dram.tile([8, 512], f16)
            nc.gpsimd.collective_compute(
                "AllToAll", OP.bypass, replica_groups=GRP,
                ins=[dumA[:].opt()], outs=[dumO[:].opt()])
            dumB = dram.tile([8, 512], f16)
            dumP = dram.tile([8, 512], f16)
            nc.gpsimd.collective_compute(
                "AllToAll", OP.bypass, replica_groups=GRP,
                ins=[dumB[:].opt()], outs=[dumP[:].opt()])

            # ---------------- persistent SBUF ----------------
            qT = [proj.tile([128, N], f16, tag=f"q{h}", name=f"qT{h}")
                  for h in range(2)]
            kT = [proj.tile([128, N], f16, tag=f"k{h}", name=f"kT{h}")
                  for h in range(2)]
            vTb = [proj.tile([128, N], f16, tag=f"v{h}", name=f"vTb{h}")
                   for h in range(2)]
            vm = [proj.tile([128, N], f16, tag=f"vm{h}", name=f"vm{h}")
                  for h in range(2)]
            vm8 = [proj.tile([128, ST, 128], f8, tag=f"vm8{h}", name=f"vm8{h}")
                   for h in range(2)]
            ybig8 = [proj.tile([128, ST, N], f8, tag=f"y{h}", name=f"ybig{h}")
                     for h in range(2)]
            # aliased slots (allocated lazily at first use): S->vm, t8snd->v,
            # g2bc->k0, t8_sb->qt
            S_sb, t8snd, g2bc_d = {}, {}, {}
            vs_sb = [cst.tile([128, 1], f16, tag=f"vs{h}", name=f"vs{h}")
                     for h in range(2)]
            vsc = [cst.tile([128, 1], f32, tag=f"vsc{h}", name=f"vsc{h}")
                   for h in range(2)]
            vsump = [cst.tile([1, C], f16, tag=f"vsump{h}", name=f"vsump{h}")
                     for h in range(2)]
            zrow = [cst.tile([1, N], f16, tag=f"zr{h}", name=f"zrow{h}")
                    for h in range(2)]
            zvw = cst.tile([16, 1536], f16)

            BI = {"q": 0, "k": 1, "v": 2}
            EV_SCALE = {"q": 1.0, "k": 1.0, "v": 1.0}
            if FP8_QK:
                EV_SCALE = {"q": SCALE * 2.0 ** (-KX - KW),
                            "k": 2.0 ** (-KX - KW), "v": 1.0}

            def proj_group(nm, h, ch, pool=None):
                """Project x @ W[nm] for head h, n-columns ch*512..+512."""
                dst = {"q": qT, "k": kT, "v": vTb}[nm][h]
                pool = pool or psc
                ps = pool.tile([128, 512], f32, tag=("raw" if pool is psr
                                                     else "c"), name="pg")
                if FP8_QK and nm in ("q", "k"):
                    w8 = {"q": wq8_sb, "k": wk8_sb}[nm]
                    for i in range(4):
                        nc.tensor.matmul(
                            ps[:],
                            w8[:, 2 * i:2 * i + 2, h * 128:(h + 1) * 128],
                            q8_sb[:, 2 * i:2 * i + 2,
                                  ch * 512:ch * 512 + 512],
                            start=(i == 0), stop=(i == 3), perf_mode=DR)
                else:
                    for ci in range(8):
                        nc.tensor.matmul(
                            ps[:],
                            w_sb[nm][:, ci * 256 + h * 128:
                                     ci * 256 + (h + 1) * 128],
                            qt_sb[:, ci * N + ch * 512:ci * N + ch * 512 + 512],
                            start=(ci == 0), stop=(ci == 7))
                nc.scalar.activation(dst[:, ch * 512:(ch + 1) * 512], ps[:],
                                     AF.Identity,
                                     bias=bqkv[:, 2 * BI[nm] + h:
                                               2 * BI[nm] + h + 1],
                                     scale=EV_SCALE[nm])

            def vm_transpose(h, st):
                ps = psy.tile([128, 128], f16, tag="yv", name="vt")
                nc.tensor.transpose(ps[:], vTb[h][:, st * 128:(st + 1) * 128],
                                    idt[:])
                nc.vector.tensor_scalar(vm[h][:, st * 128:(st + 1) * 128],
                                        ps[:], maskT[:, st:st + 1], None,
                                        OP.mult)

            def vm8_cast(h, q):
                # quarter q: s-tiles 4q..4q+3
                nc.vector.tensor_copy(vm8[h][:, 4 * q:4 * q + 4, :],
                                      vm[h][:, 512 * q:512 * (q + 1)])

            def vsum_calc(h):
                pvs = psc.tile([128, 16], f32, tag="c", name="pvs")
                for st in range(ST):
                    nc.tensor.matmul(pvs[:], vm[h][:, st * 128:(st + 1) * 128],
                                     onesc[:], start=(st == 0),
                                     stop=(st == ST - 1))
                nc.vector.tensor_copy(vs_sb[h][:], pvs[:, 0:1])

            def vs8_calc(h):
                # bias for T' from the SAME fp8 values as the Yv matmul, so
                # Yv' + vs8 = 2*YmV cancels fp8 noise exactly
                pv8 = psc.tile([128, 16], f32, tag="c", name="pv8")
                for st in range(ST):
                    nc.tensor.matmul(pv8[:], vm8[h][:, st, :], onesc8[:],
                                     start=(st == 0), stop=(st == ST - 1))
                nc.vector.tensor_copy(vsc[h][:], pv8[:, 0:1])

            def vsump_calc(h):
                for ch in range(2):
                    pvp = psc.tile([1, 512], f32, tag="c", name="pvp")
                    nc.tensor.matmul(pvp[:], vs_sb[h][:],
                                     wpo_sb[:, h * C + ch * 512:
                                            h * C + ch * 512 + 512],
                                     start=True, stop=True)
                    nc.vector.tensor_copy(vsump[h][0:1, ch * 512:(ch + 1) * 512],
                                          pvp[:])

            # ---- raw + topk step: two [128,1024] PSUM halves per s-tile ----
            def raw_step(h, st):
                pa = psr.tile([128, 1024], f32, tag="raw", name="pa")
                pb = psr.tile([128, 1024], f32, tag="raw", name="pb")
                for q in range(2):
                    nc.tensor.matmul(pa[:, q * 512:(q + 1) * 512],
                                     kT[h][:, st * 128:(st + 1) * 128],
                                     qT[h][:, q * 512:(q + 1) * 512],
                                     start=True, stop=True)
                for q in range(2):
                    nc.tensor.matmul(pb[:, q * 512:(q + 1) * 512],
                                     kT[h][:, st * 128:(st + 1) * 128],
                                     qT[h][:, 1024 + q * 512:1024 + (q + 1) * 512],
                                     start=True, stop=True)
                t16 = work.tile([128, 16], f32, tag="t16", bufs=3, name="t16")
                nc.vector.max(t16[:, 0:8], pa[:])
                nc.vector.max(t16[:, 8:16], pb[:])
                top8 = work.tile([128, 8], f32, tag="top8", bufs=3,
                                 name="top8")
                nc.vector.max(top8[:], t16[:])
                nthr = work.tile([128, 1], f32, tag="nthr", bufs=3,
                                 name="nthr")
                nc.vector.tensor_scalar(nthr[:], top8[:, K - 1:K],
                                        top8[:, K:K + 1], -0.5, OP.add,
                                        OP.mult)
                nc.scalar.activation(ybig8[h][:, st, 0:1024], pa[:], AF.Sign,
                                     bias=nthr[:], scale=1.0)
                nc.scalar.activation(ybig8[h][:, st, 1024:2048], pb[:], AF.Sign,
                                     bias=nthr[:], scale=1.0)

            cnt_ps = {}

            def counts_quad(h, ch, k4):
                # 4 DR pair-matmuls (s-tile pairs 4*k4..) for n-chunk ch
                if k4 == 0:
                    cnt_ps[h] = psc.tile([128, 512], f32, tag="c", name="pc")
                for sp in range(4 * k4, 4 * k4 + 4):
                    nc.tensor.matmul(
                        cnt_ps[h][:], om8[:, 2 * sp:2 * sp + 2, :],
                        ybig8[h][:, 2 * sp:2 * sp + 2, ch * 512:ch * 512 + 512],
                        start=(sp == 0), stop=(sp == 7), perf_mode=DR)

            def counts_evac(h, ch, cnt_sb):
                nc.vector.tensor_copy(cnt_sb[:, ch * 512:(ch + 1) * 512],
                                      cnt_ps[h][0:2, :])

            yv_ps = {}

            def yv_quad(h, ch, k4):
                if k4 == 0:
                    yv_ps[h] = psy.tile([128, 512], f32, tag="yv", name="py")
                for sp in range(4 * k4, 4 * k4 + 4):
                    nc.tensor.matmul(
                        yv_ps[h][:], vm8[h][:, 2 * sp:2 * sp + 2, :],
                        ybig8[h][:, 2 * sp:2 * sp + 2, ch * 512:ch * 512 + 512],
                        start=(sp == 0), stop=(sp == 7), perf_mode=DR)

            def yv_evac(h, ch):
                if h not in S_sb:
                    S_sb[h] = proj.tile([128, N], f16, tag=f"vm{h}",
                                        name=f"Ssb{h}")
                nc.scalar.activation(S_sb[h][:, ch * 512:(ch + 1) * 512],
                                     yv_ps[h][:], AF.Identity,
                                     bias=vsc[h][:], scale=1.0)

            def w_math(h, cnt_sb):
                """c',cm' [2,N] -> g2row (bcast to g2bc) + zrow[h]."""
                ptr = psc.tile([128, 32], f16, tag="c", name="ptr")
                for t2 in range(ST):
                    nc.tensor.transpose(ptr[:, 2 * t2:2 * t2 + 2],
                                        cnt_sb[:, t2 * 128:(t2 + 1) * 128],
                                        idt[:2, :2])
                cntT = work.tile([128, 32], f32, tag="cntT", name="cntT")
                nc.vector.tensor_copy(cntT[:], ptr[:])
                cp1 = work.tile([128, 16], f32, tag="cp1", name="cp1")
                nc.vector.tensor_scalar(cp1[:], cntT[:, 0:32:2], 0.5,
                                        float(N) / 2 + 1.0, OP.mult, OP.add)
                rec = work.tile([128, 16], f32, tag="rec", name="rec")
                nc.vector.reciprocal(rec[:], cp1[:])
                e = work.tile([128, 16], f32, tag="e", name="e")
                nc.scalar.activation(e[:], rec[:], AF.Exp)
                em1 = work.tile([128, 16], f32, tag="em1", name="em1")
                nc.vector.tensor_scalar(em1[:], e[:], -1.0, None, OP.add)
                cm = work.tile([128, 16], f32, tag="cm", name="cm")
                nc.vector.tensor_scalar(cm[:], cntT[:, 1:32:2], mbcol[:, 0:1],
                                        0.5, OP.add, OP.mult)
                Z = work.tile([128, 16], f32, tag="Z", name="Zt")
                nc.vector.tensor_mul(Z[:], em1[:], cm[:])
                nc.vector.tensor_scalar(Z[:], Z[:], mbcol[:, 0:1], None, OP.add)
                rz = work.tile([128, 16], f32, tag="rz", name="rz")
                nc.vector.reciprocal(rz[:], Z[:])
                g2 = work.tile([128, 16], f32, tag="g2", name="g2")
                nc.vector.tensor_mul(g2[:], em1[:], rz[:])
                nc.vector.tensor_scalar(g2[:], g2[:], 0.5 * 2.0 ** KT, None,
                                        OP.mult)
                z2 = work.tile([128, 16], f32, tag="z2", name="z2")
                nc.vector.tensor_scalar(z2[:], rz[:], 2.0 ** (KT + KP), None,
                                        OP.mult)
                # transpose cols -> rows
                for src, dstrow in ((g2, None), (z2, zrow[h])):
                    prt = psc.tile([16, 128], f32, tag="c", name="prt")
                    nc.tensor.transpose(prt[:], src[:], idf[:])
                    r16 = work.tile([16, 128], f16, tag="r16", name="r16")
                    nc.vector.tensor_copy(r16[:], prt[:])
                    if dstrow is None:
                        grow = work.tile([1, N], f16, tag="grow", bufs=1,
                                         name="grow")
                        nc.sync.dma_start(grow[:], r16[:])
                        bc = proj.tile([128, N], f16, tag="k0", name="g2bc")
                        g2bc_d[h] = bc
                        nc.gpsimd.partition_broadcast(bc[:], grow[:])
                    else:
                        nc.sync.dma_start(dstrow[:], r16[:])

            def tmul(h):
                t8snd[h] = proj.tile([128, N], f8, tag=f"v{h}",
                                     name=f"t8snd{h}")
                nc.vector.tensor_mul(t8snd[h][:], S_sb[h][:], g2bc_d[h][:])

            # ================= schedule =================
            # prologue: q0 (psr rotation) then k0-ch0; k0-ch1..3 ride in A0
            for ch in range(4):
                proj_group("q", 0, ch, pool=psr)
            for ch in range(4):
                proj_group("k", 0, ch, pool=psr)

            PG = {st: [(("v", "v", "q", "k")[st // 4], st // 8
                        if st < 8 else 1, st % 4)] for st in range(ST)}
            PG = {0: [("v", 0, 0)], 1: [("v", 0, 1)],
                  2: [("v", 0, 2)], 3: [("v", 0, 3)],
                  4: [("v", 1, 0)], 5: [("v", 1, 1)],
                  6: [("v", 1, 2)], 7: [("v", 1, 3)],
                  8: [("q", 1, 0)], 9: [("q", 1, 1)],
                  10: [("q", 1, 2)], 11: [("q", 1, 3)],
                  12: [("k", 1, 0)], 13: [("k", 1, 1)],
                  14: [("k", 1, 2)], 15: [("k", 1, 3)]}

            # A0: head-0 raw/topk + all remaining projections + vm0 + vm1
            for st in range(ST):
                raw_step(0, st)
                for g in PG[st]:
                    proj_group(*g)
                if 5 <= st < 13:
                    vm_transpose(0, 2 * (st - 5))
                    vm_transpose(0, 2 * (st - 5) + 1)
                if 8 <= st < 16:
                    vm_transpose(1, 2 * (st - 8))
                    vm_transpose(1, 2 * (st - 8) + 1)
                if st in (9, 10, 12, 13):
                    vm8_cast(0, {9: 0, 10: 1, 12: 2, 13: 3}[st])
                if st == 13:
                    vsum_calc(0)
                if st == 14:
                    vs8_calc(0)
                    vsump_calc(0)

            # A1: head-1 raw/topk + counts0 + yv0 + w0 + T~0 (+A2A-T0 early)
            cnt0 = work.tile([2, N], f16, tag="cnt0", bufs=1, name="cnt0")
            a2aT_in = [dram.tile([1024, 512], f8, tag=f"ati{h}",
                                 name=f"a2aTin{h}") for h in range(2)]
            a2aT_out = [dram.tile([1024, 512], f8, tag=f"ato{h}",
                                  name=f"a2aTout{h}") for h in range(2)]
            a2aZ_in = dram.tile([16, 1536], f16)
            a2aZ_out = dram.tile([16, 1536], f16)
            t8_sb = ld.tile([128, 16, 512], f8, tag="qt_sb", name="t8sb")
            for st in range(ST):
                raw_step(1, st)
                if st < 4:
                    vm8_cast(1, st)
                if 1 <= st <= 8:
                    chc, k4 = (st - 1) // 2, (st - 1) % 2
                    counts_quad(0, chc, k4)
                    if k4 == 1:
                        counts_evac(0, chc, cnt0)
                if 3 <= st <= 10:
                    chy, k4 = (st - 3) // 2, (st - 3) % 2
                    yv_quad(0, chy, k4)
                    if k4 == 1:
                        yv_evac(0, chy)
                if st == 9:
                    vsum_calc(1)
                    vs8_calc(1)
                if st == 10:
                    vsump_calc(1)
                if st == 11:
                    w_math(0, cnt0)
                    # vsump rows of the Z payload can stage now
                    for q in range(8):
                        for h in range(2):
                            QS[(2 * q + h) % 3].dma_start(
                                a2aZ_in[2 * q + h:2 * q + h + 1, 512:1536],
                                vsump[h][:])
                if st == 12:
                    # zrow[0] rows of the Z payload
                    for q in range(8):
                        QS[q % 3].dma_start(
                            a2aZ_in[2 * q:2 * q + 1, 0:512],
                            zrow[0][0:1, 512 * (q % 4):512 * (q % 4) + 512])
                if st == 13:
                    tmul(0)
                if st == 14:
                    for q in range(8):
                        QS[q % 3].dma_start(
                            a2aT_in[0][q * 128:(q + 1) * 128, :],
                            t8snd[0][:, 512 * (q % 4):512 * (q % 4) + 512])
                if st == 15:
                    nc.gpsimd.collective_compute(
                        "AllToAll", OP.bypass, replica_groups=GRP,
                        ins=[a2aT_in[0][:].opt()],
                        outs=[a2aT_out[0][:].opt()])

            # tail: counts1 + w1 + yv1 + T~1 + A2A-Z + A2A-T1 + out
            cnt1 = work.tile([2, N], f16, tag="cnt1", bufs=1, name="cnt1")
            for ch in range(4):
                counts_quad(1, ch, 0)
                counts_quad(1, ch, 1)
                counts_evac(1, ch, cnt1)
            w_math(1, cnt1)
            for ch in range(4):
                yv_quad(1, ch, 0)
                yv_quad(1, ch, 1)
                yv_evac(1, ch)

            # A2A-Z: only zrow[1] rows still need staging
            for p in range(8):
                QS[p % 3].dma_start(t8_sb[:, p, :],
                                    a2aT_out[0][p * 128:(p + 1) * 128, :])
            for q in range(8):
                QS[q % 3].dma_start(
                    a2aZ_in[2 * q + 1:2 * q + 2, 0:512],
                    zrow[1][0:1, 512 * (q % 4):512 * (q % 4) + 512])
            nc.gpsimd.collective_compute(
                "AllToAll", OP.bypass, replica_groups=GRP,
                ins=[a2aZ_in[:].opt()], outs=[a2aZ_out[:].opt()])

            tmul(1)
            for q in range(8):
                QS[q % 3].dma_start(
                    a2aT_in[1][q * 128:(q + 1) * 128, :],
                    t8snd[1][:, 512 * (q % 4):512 * (q % 4) + 512])
            nc.gpsimd.collective_compute(
                "AllToAll", OP.bypass, replica_groups=GRP,
                ins=[a2aT_in[1][:].opt()], outs=[a2aT_out[1][:].opt()])

            # receive: t8 slots j = 8h + p; zvw + group mask
            nc.sync.dma_start(zvw[:], a2aZ_out[:])
            zvwm = cst.tile([16, 1536], f16)
            nc.vector.tensor_scalar(zvwm[:], zvw[:], gmask[:, 0:1], None,
                                    OP.mult)
            for p in range(8):
                QS[p % 3].dma_start(t8_sb[:, 8 + p, :],
                                    a2aT_out[1][p * 128:(p + 1) * 128, :])

            # out tiles: [128 n, 512 c] = R + T~all^T Wp8, scale 2^-(KT+KP)
            def out_tile(cch, nt):
                ps = psr.tile([128, 512], f32, tag="raw", name="po")
                nc.tensor.matmul(ps[:], zvwm[:, nt * 128:(nt + 1) * 128],
                                 zvwm[:, 512 + cch * 512:512 + (cch + 1) * 512],
                                 start=True, stop=False)
                for i in range(8):
                    nc.tensor.matmul(
                        ps[:], t8_sb[:, 2 * i:2 * i + 2, nt * 128:(nt + 1) * 128],
                        wp8_sb[:, 2 * i:2 * i + 2, cch * 512:cch * 512 + 512],
                        start=False, stop=(i == 7), perf_mode=DR)
                ob = outp.tile([128, 512], f16, tag="ob", name="ob")
                nc.scalar.activation(ob[:], ps[:], AF.Copy, bias=0.0,
                                     scale=2.0 ** (-KT - KP))
                QS[(2 * cch + nt) % 3].dma_start(
                    d_out[nt * 128:(nt + 1) * 128,
                          cch * 512:(cch + 1) * 512], ob[:])

            for nt in range(4):
                for cch in range(2):
                    out_tile(cch, nt)

    nc.compile()
    return nc


def _host_inputs(query, mask, Wq, bq, Wk, bk, Wv, bv, Wp, bp):
    f16 = np.float16
    ins = []
    idt = np.eye(128, dtype=f16)
    idf = np.eye(128, dtype=np.float32)
    onesc = np.ones((128, 16), dtype=f16)
    onesc8 = np.ones((128, 16), dtype=f8t)
    Wp8full = (np.asarray(Wp, np.float32) * 2.0 ** KP).astype(f8t)
    for c in range(NCORES):
        b, r = c // 4, c % 4
        h0 = 2 * r
        sl = slice(h0 * HD, (h0 + 2) * HD)
        qt = np.ascontiguousarray(np.asarray(query[b]).T).astype(f16)
        maskT = np.ascontiguousarray(
            np.asarray(mask[b]).reshape(ST, 128).T.astype(np.float32))
        om8 = np.zeros((128, ST, 128), dtype=f8t)
        om8[:, :, 0] = 1.0
        om8[:, :, 1] = maskT.astype(f8t)
        mbcol = np.full((128, 1), float(np.asarray(mask[b]).sum()),
                        dtype=np.float32)
        bqkv = np.zeros((128, 6), dtype=np.float32)
        for i in range(2):
            bqkv[:, 0 + i] = SCALE * np.asarray(bq)[(h0 + i) * HD:
                                                    (h0 + i + 1) * HD]
            bqkv[:, 2 + i] = np.asarray(bk)[(h0 + i) * HD:(h0 + i + 1) * HD]
            bqkv[:, 4 + i] = np.asarray(bv)[(h0 + i) * HD:(h0 + i + 1) * HD]
        # wpo: [hd, h, c] own-head Wp rows (f16)
        wpo = np.ascontiguousarray(
            np.asarray(Wp, np.float32)[sl, :].reshape(2, HD, C)
            .transpose(1, 0, 2).reshape(HD, 2 * C)).astype(f16)
        # wp8: [hd, slot j = 8h + p, c], zero for wrong-group p
        wp8 = np.zeros((HD, 16, C), dtype=f8t)
        for h in range(2):
            for p in range(8):
                if p // 4 == b:
                    g = 2 * (p % 4) + h
                    wp8[:, 8 * h + p, :] = Wp8full[g * HD:(g + 1) * HD, :]
        wp8 = wp8.reshape(HD, 16 * C)
        gmask = np.zeros((16, 1), dtype=np.float32)
        for h in range(2):
            for p in range(8):
                if p // 4 == b:
                    gmask[2 * p + h, 0] = 1.0
        d = dict(
            qt=qt,
            wq=np.ascontiguousarray(np.asarray(Wq)[:, sl] * SCALE).astype(f16),
            wk=np.ascontiguousarray(np.asarray(Wk)[:, sl]).astype(f16),
            wv=np.ascontiguousarray(np.asarray(Wv)[:, sl]).astype(f16),
            wpo=wpo, wp8=wp8, maskT=maskT, om8=om8, mbcol=mbcol,
            onesc=onesc, onesc8=onesc8, idt=idt, idf=idf, bqkv=bqkv,
            gmask=gmask)
        if FP8_QK:
            d["q8"] = (qt.astype(np.float32) * 2.0 ** KX).astype(f8t)
            d["wq8"] = np.ascontiguousarray(
                (np.asarray(Wq, np.float32)[:, sl] * 2.0 ** KW)
                .reshape(8, 128, 2 * HD).transpose(1, 0, 2)).astype(f8t)
            d["wk8"] = np.ascontiguousarray(
                (np.asarray(Wk, np.float32)[:, sl] * 2.0 ** KW)
                .reshape(8, 128, 2 * HD).transpose(1, 0, 2)).astype(f8t)
        ins.append(d)
    return ins


def kernel(query, mask, Wq, bq, Wk, bk, Wv, bv, Wp, bp):
    from concourse.bass_utils import run_bass_kernel_spmd

    if "nc" not in _cache:
        _cache["nc"] = _build()
    nc = _cache["nc"]
    ins = _host_inputs(query, mask, Wq, bq, Wk, bk, Wv, bv, Wp, bp)
    res = run_bass_kernel_spmd(nc, ins, list(range(NCORES)))
    out = np.empty((B, N, C), dtype=np.float32)
    for b in range(B):
        for r in range(4):
            o = res.results[4 * b + r]["out"].astype(np.float32)
            out[b, 512 * r:512 * (r + 1)] = o
    out += np.asarray(bp, dtype=np.float32)[None, None, :]
    return out


# revision 17
# speedup vs baseline: 1.1368x; 1.1368x over previous
"""AssignAttention (topk_masking) Trainium2 kernel v2 — 8 NeuronCores.

Sharding: data-parallel over B (2 groups of 4 cores), tensor-parallel over
heads H (2 heads per core).

Math (per head): Y[n,s] = [n in top4 of column s of raw scores],
c_n = sum_s Y, cm_n = sum_s mask_s Y, e_n = exp(1/(c_n+1)), M = sum_s mask_s,
Z_n = M + (e_n-1) cm_n. Then
  out_head[n,:] = Vsum/Z_n + ((e_n-1)/Z_n) * (Y.mask @ v)[n,:]  (@ Wp rows)
which equals hard-topk + sum-normalize + masked softmax (see reference).

v2 structure vs baseline:
- Y' in {-1,+1} produced by Act-engine Sign((raw - midpoint(top4,top5))),
  with MAX8 reading raw straight from PSUM (no PSUM->SBUF evac of raw).
  Fixups: c = (c'+N)/2, cm = (cm'+M)/2, YmV = (Yv'+vs)/2 folded into the
  scalar math and the rank-one R term.
- counts matmuls run fp8 DoubleRow (lhsT padded to 128 wide).
- Final output: per-core n-slice via an 8-core AllToAll of the scaled
  T~ = (e-1)/(2Z) * Yv' matrices (fp8) + a tiny AllToAll of z/vsump rows;
  each core then computes out[nslice] = z2^T vsump + T~all^T Wp8 locally.
  No ReduceScatter chain.
"""
import sys, os

os.environ["JAX_ENABLE_COMPILATION_CACHE"] = "false"
sys.path.insert(0, "/opt/trn_rl_repo")
import numpy as np
import ml_dtypes

B, N, C, H, K = 2, 2048, 1024, 8, 4
HD = C // H
SCALE = HD ** -0.5
NCORES = 8
ST = 16            # 128-row s-tiles per head
KT = 12            # T~ fp8 prescale exponent
KP = 7             # Wp fp8 prescale exponent
KX = 6             # x fp8 prescale exponent (fp8 qk path)
KW = 12            # Wq/Wk fp8 prescale exponent
FP8_QK = False

f8t = ml_dtypes.float8_e4m3fn
_cache = {}


def _build():
    from concourse import bacc, tile, mybir

    f32, f16 = mybir.dt.float32, mybir.dt.float16
    f8 = mybir.dt.float8e4
    DR = mybir.MatmulPerfMode.DoubleRow
    AF = mybir.ActivationFunctionType
    OP = mybir.AluOpType
    GRP = [[0, 1, 2, 3, 4, 5, 6, 7]]

    nc = bacc.Bacc(None, target_bir_lowering=False)
    d_qt = nc.declare_dram_parameter("qt", [C, N], f16, isOutput=False)
    d_wq = nc.declare_dram_parameter("wq", [C, 2 * HD], f16, isOutput=False)
    d_wk = nc.declare_dram_parameter("wk", [C, 2 * HD], f16, isOutput=False)
    d_wv = nc.declare_dram_parameter("wv", [C, 2 * HD], f16, isOutput=False)
    d_wpo = nc.declare_dram_parameter("wpo", [128, 2 * C], f16, isOutput=False)
    d_wp8 = nc.declare_dram_parameter("wp8", [128, 16 * C], f8, isOutput=False)
    d_maskT = nc.declare_dram_parameter("maskT", [128, ST], f32, isOutput=False)
    d_om8 = nc.declare_dram_parameter("om8", [128, ST, 128], f8, isOutput=False)
    d_mb = nc.declare_dram_parameter("mbcol", [128, 1], f32, isOutput=False)
    d_ones = nc.declare_dram_parameter("onesc", [128, 16], f16, isOutput=False)
    d_ones8 = nc.declare_dram_parameter("onesc8", [128, 16], f8, isOutput=False)
    d_idt = nc.declare_dram_parameter("idt", [128, 128], f16, isOutput=False)
    d_idf = nc.declare_dram_parameter("idf", [128, 128], f32, isOutput=False)
    d_bqkv = nc.declare_dram_parameter("bqkv", [128, 6], f32, isOutput=False)
    d_gmask = nc.declare_dram_parameter("gmask", [16, 1], f32, isOutput=False)
    if FP8_QK:
        d_q8 = nc.declare_dram_parameter("q8", [C, N], f8, isOutput=False)
        d_wq8 = nc.declare_dram_parameter("wq8", [128, 8, 2 * HD], f8,
                                          isOutput=False)
        d_wk8 = nc.declare_dram_parameter("wk8", [128, 8, 2 * HD], f8,
                                          isOutput=False)
    d_out = nc.declare_dram_parameter("out", [N // 4, C], f16, isOutput=True)

    with tile.TileContext(nc) as tc:
        with (
            tc.tile_pool(name="cst", bufs=1) as cst,
            tc.tile_pool(name="ld", bufs=1) as ld,
            tc.tile_pool(name="proj", bufs=1) as proj,
            tc.tile_pool(name="work", bufs=2) as work,
            tc.tile_pool(name="outp", bufs=2) as outp,
            tc.tile_pool(name="psr", bufs=6, space="PSUM") as psr,
            tc.tile_pool(name="psc", bufs=1, space="PSUM") as psc,
            tc.tile_pool(name="psy", bufs=1, space="PSUM") as psy,
            tc.tile_pool(name="dram", bufs=1, space="DRAM") as dram,
        ):
            QS = [nc.sync, nc.scalar, nc.gpsimd]

            # ---------------- input DMAs ----------------
            # consts first (tiny), then qt chunks + weights, tail weights last
            if FP8_QK:
                q8_sb = ld.tile([128, 8, N], f8)
                for ci in range(8):
                    QS[ci % 3].dma_start(q8_sb[:, ci, :],
                                         d_q8[ci * 128:(ci + 1) * 128, :])
                wq8_sb = ld.tile([128, 8, 256], f8)
                wk8_sb = ld.tile([128, 8, 256], f8)
                nc.sync.dma_start(wq8_sb[:], d_wq8[:])
                nc.scalar.dma_start(wk8_sb[:], d_wk8[:])
            maskT = cst.tile([128, ST], f32)
            om8 = cst.tile([128, ST, 128], f8)
            mbcol = cst.tile([128, 1], f32)
            onesc = cst.tile([128, 16], f16)
            onesc8 = cst.tile([128, 16], f8)
            idt = cst.tile([128, 128], f16)
            idf = cst.tile([128, 128], f32)
            bqkv = cst.tile([128, 6], f32)
            gmask = cst.tile([16, 1], f32)
            for i, (t, d) in enumerate([(maskT, d_maskT), (mbcol, d_mb),
                                        (onesc, d_ones), (onesc8, d_ones8),
                                        (idt, d_idt),
                                        (idf, d_idf), (bqkv, d_bqkv),
                                        (gmask, d_gmask)]):
                QS[i % 3].dma_start(t[:], d[:])
            qt_sb = ld.tile([128, 8 * N], f16)
            for ci in range(8):
                for j in range(4):
                    QS[(4 * ci + j) % 3].dma_start(
                        qt_sb[:, ci * N + j * 512:ci * N + (j + 1) * 512],
                        d_qt[ci * 128:(ci + 1) * 128,
                             j * 512:(j + 1) * 512])
            w_sb = {}
            for nm, dd in (("q", d_wq), ("k", d_wk), ("v", d_wv)):
                w_sb[nm] = ld.tile([128, 8 * 2 * HD], f16, tag=f"w{nm}",
                                   name=f"wsb{nm}")
                for ci in range(8):
                    QS[(ci + 1) % 3].dma_start(
                        w_sb[nm][:, ci * 256:(ci + 1) * 256],
                        dd[ci * 128:(ci + 1) * 128, :])
            nc.sync.dma_start(om8[:], d_om8[:])
            wpo_sb = cst.tile([128, 2 * C], f16)
            nc.gpsimd.dma_start(wpo_sb[:], d_wpo[:])
            wp8_sb = cst.tile([128, 16, C], f8)
            for i in range(4):
                QS[i % 3].dma_start(wp8_sb[:, 4 * i:4 * i + 4, :],
                                    d_wp8[:, 4 * i * C:(4 * i + 4) * C])

            # ---------------- collective warmup ----------------
            dumA = dram.tile([8, 512], f16)
            dumO = dram.tile([8, 512], f16)
            nc.gpsimd.collective_compute(
                "AllToAll", OP.bypass, replica_groups=GRP,
                ins=[dumA[:].opt()], outs=[dumO[:].opt()])
            dumB = dram.tile([8, 512], f16)
            dumP = dram.tile([8, 512], f16)
            nc.gpsimd.collective_compute(
                "AllToAll", OP.bypass, replica_groups=GRP,
                ins=[dumB[:].opt()], outs=[dumP[:].opt()])

            # ---------------- persistent SBUF ----------------
            qT = [proj.tile([128, N], f16, tag=f"q{h}", name=f"qT{h}")
                  for h in range(2)]
            kT = [proj.tile([128, N], f16, tag=f"k{h}", name=f"kT{h}")
                  for h in range(2)]
            vTb = [proj.tile([128, N], f16, tag=f"v{h}", name=f"vTb{h}")
                   for h in range(2)]
            vm = [proj.tile([128, N], f16, tag=f"vm{h}", name=f"vm{h}")
                  for h in range(2)]
            vm8 = [proj.tile([128, ST, 128], f8, tag=f"vm8{h}", name=f"vm8{h}")
                   for h in range(2)]
            ybig8 = [proj.tile([128, ST, N], f8, tag=f"y{h}", name=f"ybig{h}")
                     for h in range(2)]
            # aliased slots (allocated lazily at first use): S->vm, t8snd->v,
            # g2bc->k0, t8_sb->qt
            S_sb, t8snd, g2bc_d = {}, {}, {}
            vs_sb = [cst.tile([128, 1], f16, tag=f"vs{h}", name=f"vs{h}")
                     for h in range(2)]
            vsc = [cst.tile([128, 1], f32, tag=f"vsc{h}", name=f"vsc{h}")
                   for h in range(2)]
            vsump = [cst.tile([1, C], f16, tag=f"vsump{h}", name=f"vsump{h}")
                     for h in range(2)]
            zrow = [cst.tile([1, N], f16, tag=f"zr{h}", name=f"zrow{h}")
                    for h in range(2)]
            zvw = cst.tile([16, 1536], f16)

            BI = {"q": 0, "k": 1, "v": 2}
            EV_SCALE = {"q": 1.0, "k": 1.0, "v": 1.0}
            if FP8_QK:
                EV_SCALE = {"q": SCALE * 2.0 ** (-KX - KW),
                            "k": 2.0 ** (-KX - KW), "v": 1.0}

            def proj_group(nm, h, ch, pool=None):
                """Project x @ W[nm] for head h, n-columns ch*512..+512."""
                dst = {"q": qT, "k": kT, "v": vTb}[nm][h]
                pool = pool or psc
                ps = pool.tile([128, 512], f32, tag=("raw" if pool is psr
                                                     else "c"), name="pg")
                if FP8_QK and nm in ("q", "k"):
                    w8 = {"q": wq8_sb, "k": wk8_sb}[nm]
                    for i in range(4):
                        nc.tensor.matmul(
                            ps[:],
                            w8[:, 2 * i:2 * i + 2, h * 128:(h + 1) * 128],
                            q8_sb[:, 2 * i:2 * i + 2,
                                  ch * 512:ch * 512 + 512],
                            start=(i == 0), stop=(i == 3), perf_mode=DR)
                else:
                    for ci in range(8):
                        nc.tensor.matmul(
                            ps[:],
                            w_sb[nm][:, ci * 256 + h * 128:
                                     ci * 256 + (h + 1) * 128],
                            qt_sb[:, ci * N + ch * 512:ci * N + ch * 512 + 512],
                            start=(ci == 0), stop=(ci == 7))
                nc.scalar.activation(dst[:, ch * 512:(ch + 1) * 512], ps[:],
                                     AF.Identity,
                                     bias=bqkv[:, 2 * BI[nm] + h:
                                               2 * BI[nm] + h + 1],
                                     scale=EV_SCALE[nm])

            def vm_transpose(h, st):
                ps = psy.tile([128, 128], f16, tag="yv", name="vt")
                nc.tensor.transpose(ps[:], vTb[h][:, st * 128:(st + 1) * 128],
                                    idt[:])
                nc.vector.tensor_scalar(vm[h][:, st * 128:(st + 1) * 128],
                                        ps[:], maskT[:, st:st + 1], None,
                                        OP.mult)

            def vm8_cast(h, q):
                # quarter q: s-tiles 4q..4q+3
                nc.vector.tensor_copy(vm8[h][:, 4 * q:4 * q + 4, :],
                                      vm[h][:, 512 * q:512 * (q + 1)])

            def vsum_calc(h):
                pvs = psc.tile([128, 16], f32, tag="c", name="pvs")
                for st in range(ST):
                    nc.tensor.matmul(pvs[:], vm[h][:, st * 128:(st + 1) * 128],
                                     onesc[:], start=(st == 0),
                                     stop=(st == ST - 1))
                nc.vector.tensor_copy(vs_sb[h][:], pvs[:, 0:1])

            def vs8_calc(h):
                # bias for T' from the SAME fp8 values as the Yv matmul, so
                # Yv' + vs8 = 2*YmV cancels fp8 noise exactly
                pv8 = psc.tile([128, 16], f32, tag="c", name="pv8")
                for st in range(ST):
                    nc.tensor.matmul(pv8[:], vm8[h][:, st, :], onesc8[:],
                                     start=(st == 0), stop=(st == ST - 1))
                nc.vector.tensor_copy(vsc[h][:], pv8[:, 0:1])

            def vsump_calc(h):
                for ch in range(2):
                    pvp = psc.tile([1, 512], f32, tag="c", name="pvp")
                    nc.tensor.matmul(pvp[:], vs_sb[h][:],
                                     wpo_sb[:, h * C + ch * 512:
                                            h * C + ch * 512 + 512],
                                     start=True, stop=True)
                    nc.vector.tensor_copy(vsump[h][0:1, ch * 512:(ch + 1) * 512],
                                          pvp[:])

            # ---- raw + topk step: four [128,512] PSUM quarters/tile ----
            def raw_step(h, st):
                ps = [psr.tile([128, 512], f32, tag="raw", name=f"pq{q}")
                      for q in range(4)]
                for q in range(4):
                    nc.tensor.matmul(ps[q][:],
                                     kT[h][:, st * 128:(st + 1) * 128],
                                     qT[h][:, q * 512:(q + 1) * 512],
                                     start=True, stop=True)
                t32 = work.tile([128, 32], f32, tag="t32", bufs=3, name="t32")
                for q in range(4):
                    nc.vector.max(t32[:, 8 * q:8 * q + 8], ps[q][:])
                top8 = work.tile([128, 8], f32, tag="top8", bufs=3,
                                 name="top8")
                nc.vector.max(top8[:], t32[:])
                nthr = work.tile([128, 1], f32, tag="nthr", bufs=3,
                                 name="nthr")
                nc.vector.tensor_scalar(nthr[:], top8[:, K - 1:K],
                                        top8[:, K:K + 1], -0.5, OP.add,
                                        OP.mult)
                for q in range(4):
                    nc.scalar.activation(
                        ybig8[h][:, st, q * 512:(q + 1) * 512], ps[q][:],
                        AF.Sign, bias=nthr[:], scale=1.0)

            cnt_ps = {}

            def counts_quad(h, ch, k4):
                # 4 DR pair-matmuls (s-tile pairs 4*k4..) for n-chunk ch
                if k4 == 0:
                    cnt_ps[h] = psc.tile([128, 512], f32, tag="c", name="pc")
                for sp in range(4 * k4, 4 * k4 + 4):
                    nc.tensor.matmul(
                        cnt_ps[h][:], om8[:, 2 * sp:2 * sp + 2, :],
                        ybig8[h][:, 2 * sp:2 * sp + 2, ch * 512:ch * 512 + 512],
                        start=(sp == 0), stop=(sp == 7), perf_mode=DR)

            def counts_evac(h, ch, cnt_sb):
                nc.vector.tensor_copy(cnt_sb[:, ch * 512:(ch + 1) * 512],
                                      cnt_ps[h][0:2, :])

            yv_ps = {}

            def yv_quad(h, ch, k4):
                if k4 == 0:
                    yv_ps[h] = psy.tile([128, 512], f32, tag="yv", name="py")
                for sp in range(4 * k4, 4 * k4 + 4):
                    nc.tensor.matmul(
                        yv_ps[h][:], vm8[h][:, 2 * sp:2 * sp + 2, :],
                        ybig8[h][:, 2 * sp:2 * sp + 2, ch * 512:ch * 512 + 512],
                        start=(sp == 0), stop=(sp == 7), perf_mode=DR)

            def yv_evac(h, ch):
                if h not in S_sb:
                    S_sb[h] = proj.tile([128, N], f16, tag=f"vm{h}",
                                        name=f"Ssb{h}")
                nc.scalar.activation(S_sb[h][:, ch * 512:(ch + 1) * 512],
                                     yv_ps[h][:], AF.Identity,
                                     bias=vsc[h][:], scale=1.0)

            def w_math(h, cnt_sb):
                """c',cm' [2,N] -> g2row (bcast to g2bc) + zrow[h]."""
                ptr = psc.tile([128, 32], f16, tag="c", name="ptr")
                for t2 in range(ST):
                    nc.tensor.transpose(ptr[:, 2 * t2:2 * t2 + 2],
                                        cnt_sb[:, t2 * 128:(t2 + 1) * 128],
                                        idt[:2, :2])
                cntT = work.tile([128, 32], f32, tag="cntT", name="cntT")
                nc.vector.tensor_copy(cntT[:], ptr[:])
                cp1 = work.tile([128, 16], f32, tag="cp1", name="cp1")
                nc.vector.tensor_scalar(cp1[:], cntT[:, 0:32:2], 0.5,
                                        float(N) / 2 + 1.0, OP.mult, OP.add)
                rec = work.tile([128, 16], f32, tag="rec", name="rec")
                nc.vector.reciprocal(rec[:], cp1[:])
                e = work.tile([128, 16], f32, tag="e", name="e")
                nc.scalar.activation(e[:], rec[:], AF.Exp)
                em1 = work.tile([128, 16], f32, tag="em1", name="em1")
                nc.vector.tensor_scalar(em1[:], e[:], -1.0, None, OP.add)
                cm = work.tile([128, 16], f32, tag="cm", name="cm")
                nc.vector.tensor_scalar(cm[:], cntT[:, 1:32:2], mbcol[:, 0:1],
                                        0.5, OP.add, OP.mult)
                Z = work.tile([128, 16], f32, tag="Z", name="Zt")
                nc.vector.tensor_mul(Z[:], em1[:], cm[:])
                nc.vector.tensor_scalar(Z[:], Z[:], mbcol[:, 0:1], None, OP.add)
                rz = work.tile([128, 16], f32, tag="rz", name="rz")
                nc.vector.reciprocal(rz[:], Z[:])
                g2 = work.tile([128, 16], f32, tag="g2", name="g2")
                nc.vector.tensor_mul(g2[:], em1[:], rz[:])
                nc.vector.tensor_scalar(g2[:], g2[:], 0.5 * 2.0 ** KT, None,
                                        OP.mult)
                z2 = work.tile([128, 16], f32, tag="z2", name="z2")
                nc.vector.tensor_scalar(z2[:], rz[:], 2.0 ** (KT + KP), None,
                                        OP.mult)
                # transpose cols -> rows
                for src, dstrow in ((g2, None), (z2, zrow[h])):
                    prt = psc.tile([16, 128], f32, tag="c", name="prt")
                    nc.tensor.transpose(prt[:], src[:], idf[:])
                    r16 = work.tile([16, 128], f16, tag="r16", name="r16")
                    nc.vector.tensor_copy(r16[:], prt[:])
                    if dstrow is None:
                        grow = work.tile([1, N], f16, tag="grow", bufs=1,
                                         name="grow")
                        nc.sync.dma_start(grow[:], r16[:])
                        bc = proj.tile([128, N], f16, tag="k0", name="g2bc")
                        g2bc_d[h] = bc
                        nc.gpsimd.partition_broadcast(bc[:], grow[:])
                    else:
                        nc.sync.dma_start(dstrow[:], r16[:])

            def tmul(h):
                t8snd[h] = proj.tile([128, N], f8, tag=f"v{h}",
                                     name=f"t8snd{h}")
                nc.vector.tensor_mul(t8snd[h][:], S_sb[h][:], g2bc_d[h][:])

            # ================= schedule =================
            # prologue: q0 (psr rotation) then k0-ch0; k0-ch1..3 ride in A0
            for ch in range(4):
                proj_group("q", 0, ch, pool=psr)
            for ch in range(4):
                proj_group("k", 0, ch, pool=psr)

            PG = {st: [(("v", "v", "q", "k")[st // 4], st // 8
                        if st < 8 else 1, st % 4)] for st in range(ST)}
            PG = {0: [("v", 0, 0)], 1: [("v", 0, 1)],
                  2: [("v", 0, 2)], 3: [("v", 0, 3)],
                  4: [("v", 1, 0)], 5: [("v", 1, 1)],
                  6: [("v", 1, 2)], 7: [("v", 1, 3)],
                  8: [("q", 1, 0)], 9: [("q", 1, 1)],
                  10: [("q", 1, 2)], 11: [("q", 1, 3)],
                  12: [("k", 1, 0)], 13: [("k", 1, 1)],
                  14: [("k", 1, 2)], 15: [("k", 1, 3)]}

            # A0: head-0 raw/topk + all remaining projections + vm0 + vm1
            for st in range(ST):
                raw_step(0, st)
                for g in PG[st]:
                    proj_group(*g)
                if 5 <= st < 13:
                    vm_transpose(0, 2 * (st - 5))
                    vm_transpose(0, 2 * (st - 5) + 1)
                if 8 <= st < 16:
                    vm_transpose(1, 2 * (st - 8))
                    vm_transpose(1, 2 * (st - 8) + 1)
                if st in (9, 10, 12, 13):
                    vm8_cast(0, {9: 0, 10: 1, 12: 2, 13: 3}[st])
                if st == 13:
                    vsum_calc(0)
                if st == 14:
                    vs8_calc(0)
                    vsump_calc(0)

            # A1: head-1 raw/topk + counts0 + yv0 + w0 + T~0 (+A2A-T0 early)
            cnt0 = work.tile([2, N], f16, tag="cnt0", bufs=1, name="cnt0")
            a2aT_in = [dram.tile([1024, 512], f8, tag=f"ati{h}",
                                 name=f"a2aTin{h}") for h in range(2)]
            a2aT_out = [dram.tile([1024, 512], f8, tag=f"ato{h}",
                                  name=f"a2aTout{h}") for h in range(2)]
            a2aZ_in = dram.tile([16, 1536], f16)
            a2aZ_out = dram.tile([16, 1536], f16)
            t8_sb = ld.tile([128, 16, 512], f8, tag="qt_sb", name="t8sb")
            for st in range(ST):
                raw_step(1, st)
                if st < 4:
                    vm8_cast(1, st)
                if 1 <= st <= 8:
                    chc, k4 = (st - 1) // 2, (st - 1) % 2
                    counts_quad(0, chc, k4)
                    if k4 == 1:
                        counts_evac(0, chc, cnt0)
                if 3 <= st <= 10:
                    chy, k4 = (st - 3) // 2, (st - 3) % 2
                    yv_quad(0, chy, k4)
                    if k4 == 1:
                        yv_evac(0, chy)
                if st == 9:
                    vsum_calc(1)
                    vs8_calc(1)
                if st == 10:
                    vsump_calc(1)
                if st == 11:
                    w_math(0, cnt0)
                    # vsump rows of the Z payload can stage now
                    for q in range(8):
                        for h in range(2):
                            QS[(2 * q + h) % 3].dma_start(
                                a2aZ_in[2 * q + h:2 * q + h + 1, 512:1536],
                                vsump[h][:])
                if st == 12:
                    # zrow[0] rows of the Z payload
                    for q in range(8):
                        QS[q % 3].dma_start(
                            a2aZ_in[2 * q:2 * q + 1, 0:512],
                            zrow[0][0:1, 512 * (q % 4):512 * (q % 4) + 512])
                if st == 13:
                    tmul(0)
                if st == 14:
                    for q in range(8):
                        QS[q % 3].dma_start(
                            a2aT_in[0][q * 128:(q + 1) * 128, :],
                            t8snd[0][:, 512 * (q % 4):512 * (q % 4) + 512])
                if st == 15:
                    nc.gpsimd.collective_compute(
                        "AllToAll", OP.bypass, replica_groups=GRP,
                        ins=[a2aT_in[0][:].opt()],
                        outs=[a2aT_out[0][:].opt()])

            # tail: counts1 + w1 + yv1 + T~1 + A2A-Z + A2A-T1 + out
            cnt1 = work.tile([2, N], f16, tag="cnt1", bufs=1, name="cnt1")
            for ch in range(4):
                counts_quad(1, ch, 0)
                counts_quad(1, ch, 1)
                counts_evac(1, ch, cnt1)
            w_math(1, cnt1)
            for ch in range(4):
                yv_quad(1, ch, 0)
                yv_quad(1, ch, 1)
                yv_evac(1, ch)

            # A2A-Z: only zrow[1] rows still need staging
            for p in range(8):
                QS[p % 3].dma_start(t8_sb[:, p, :],
                                    a2aT_out[0][p * 128:(p + 1) * 128, :])
            for q in range(8):
                QS[q % 3].dma_start(
                    a2aZ_in[2 * q + 1:2 * q + 2, 0:512],
                    zrow[1][0:1, 512 * (q % 4):512 * (q % 4) + 512])
            nc.gpsimd.collective_compute(
                "AllToAll", OP.bypass, replica_groups=GRP,
                ins=[a2aZ_in[:].opt()], outs=[a2aZ_out[:].opt()])

            tmul(1)
            for q in range(8):
                QS[q % 3].dma_start(
                    a2aT_in[1][q * 128:(q + 1) * 128, :],
                    t8snd[1][:, 512 * (q % 4):512 * (q % 4) + 512])
            nc.gpsimd.collective_compute(
                "AllToAll", OP.bypass, replica_groups=GRP,
                ins=[a2aT_in[1][:].opt()], outs=[a2aT_out[1][:].opt()])

            # receive: t8 slots j = 8h + p; zvw + group mask
            nc.sync.dma_start(zvw[:], a2aZ_out[:])
            zvwm = cst.tile([16, 1536], f16)
            nc.vector.tensor_scalar(zvwm[:], zvw[:], gmask[:, 0:1], None,
                                    OP.mult)
            for p in range(8):
                QS[p % 3].dma_start(t8_sb[:, 8 + p, :],
                                    a2aT_out[1][p * 128:(p + 1) * 128, :])

            # out tiles: [128 n, 512 c] = R + T~all^T Wp8, scale 2^-(KT+KP)
            def out_tile(cch, nt):
                ps = psr.tile([128, 512], f32, tag="raw", name="po")
                nc.tensor.matmul(ps[:], zvwm[:, nt * 128:(nt + 1) * 128],
                                 zvwm[:, 512 + cch * 512:512 + (cch + 1) * 512],
                                 start=True, stop=False)
                for i in range(8):
                    nc.tensor.matmul(
                        ps[:], t8_sb[:, 2 * i:2 * i + 2, nt * 128:(nt + 1) * 128],
                        wp8_sb[:, 2 * i:2 * i + 2, cch * 512:cch * 512 + 512],
                        start=False, stop=(i == 7), perf_mode=DR)
                ob = outp.tile([128, 512], f16, tag="ob", name="ob")
                nc.scalar.activation(ob[:], ps[:], AF.Copy, bias=0.0,
                                     scale=2.0 ** (-KT - KP))
                QS[(2 * cch + nt) % 3].dma_start(
                    d_out[nt * 128:(nt + 1) * 128,
                          cch * 512:(cch + 1) * 512], ob[:])

            for nt in range(4):
                for cch in range(2):
                    out_tile(cch, nt)

    nc.compile()
    return nc


def _host_inputs(query, mask, Wq, bq, Wk, bk, Wv, bv, Wp, bp):
    f16 = np.float16
    ins = []
    idt = np.eye(128, dtype=f16)
    idf = np.eye(128, dtype=np.float32)
    onesc = np.ones((128, 16), dtype=f16)
    onesc8 = np.ones((128, 16), dtype=f8t)
    Wp8full = (np.asarray(Wp, np.float32) * 2.0 ** KP).astype(f8t)
    for c in range(NCORES):
        b, r = c // 4, c % 4
        h0 = 2 * r
        sl = slice(h0 * HD, (h0 + 2) * HD)
        qt = np.ascontiguousarray(np.asarray(query[b]).T).astype(f16)
        maskT = np.ascontiguousarray(
            np.asarray(mask[b]).reshape(ST, 128).T.astype(np.float32))
        om8 = np.zeros((128, ST, 128), dtype=f8t)
        om8[:, :, 0] = 1.0
        om8[:, :, 1] = maskT.astype(f8t)
        mbcol = np.full((128, 1), float(np.asarray(mask[b]).sum()),
                        dtype=np.float32)
        bqkv = np.zeros((128, 6), dtype=np.float32)
        for i in range(2):
            bqkv[:, 0 + i] = SCALE * np.asarray(bq)[(h0 + i) * HD:
                                                    (h0 + i + 1) * HD]
            bqkv[:, 2 + i] = np.asarray(bk)[(h0 + i) * HD:(h0 + i + 1) * HD]
            bqkv[:, 4 + i] = np.asarray(bv)[(h0 + i) * HD:(h0 + i + 1) * HD]
        # wpo: [hd, h, c] own-head Wp rows (f16)
        wpo = np.ascontiguousarray(
            np.asarray(Wp, np.float32)[sl, :].reshape(2, HD, C)
            .transpose(1, 0, 2).reshape(HD, 2 * C)).astype(f16)
        # wp8: [hd, slot j = 8h + p, c], zero for wrong-group p
        wp8 = np.zeros((HD, 16, C), dtype=f8t)
        for h in range(2):
            for p in range(8):
                if p // 4 == b:
                    g = 2 * (p % 4) + h
                    wp8[:, 8 * h + p, :] = Wp8full[g * HD:(g + 1) * HD, :]
        wp8 = wp8.reshape(HD, 16 * C)
        gmask = np.zeros((16, 1), dtype=np.float32)
        for h in range(2):
            for p in range(8):
                if p // 4 == b:
                    gmask[2 * p + h, 0] = 1.0
        d = dict(
            qt=qt,
            wq=np.ascontiguousarray(np.asarray(Wq)[:, sl] * SCALE).astype(f16),
            wk=np.ascontiguousarray(np.asarray(Wk)[:, sl]).astype(f16),
            wv=np.ascontiguousarray(np.asarray(Wv)[:, sl]).astype(f16),
            wpo=wpo, wp8=wp8, maskT=maskT, om8=om8, mbcol=mbcol,
            onesc=onesc, onesc8=onesc8, idt=idt, idf=idf, bqkv=bqkv,
            gmask=gmask)
        if FP8_QK:
            d["q8"] = (qt.astype(np.float32) * 2.0 ** KX).astype(f8t)
            d["wq8"] = np.ascontiguousarray(
                (np.asarray(Wq, np.float32)[:, sl] * 2.0 ** KW)
                .reshape(8, 128, 2 * HD).transpose(1, 0, 2)).astype(f8t)
            d["wk8"] = np.ascontiguousarray(
                (np.asarray(Wk, np.float32)[:, sl] * 2.0 ** KW)
                .reshape(8, 128, 2 * HD).transpose(1, 0, 2)).astype(f8t)
        ins.append(d)
    return ins


def kernel(query, mask, Wq, bq, Wk, bk, Wv, bv, Wp, bp):
    from concourse.bass_utils import run_bass_kernel_spmd

    if "nc" not in _cache:
        _cache["nc"] = _build()
    nc = _cache["nc"]
    ins = _host_inputs(query, mask, Wq, bq, Wk, bk, Wv, bv, Wp, bp)
    res = run_bass_kernel_spmd(nc, ins, list(range(NCORES)))
    out = np.empty((B, N, C), dtype=np.float32)
    for b in range(B):
        for r in range(4):
            o = res.results[4 * b + r]["out"].astype(np.float32)
            out[b, 512 * r:512 * (r + 1)] = o
    out += np.asarray(bp, dtype=np.float32)[None, None, :]
    return out
